# revision 1
# baseline (speedup 1.0000x reference)
"""Caser query encoder on 8 Trainium2 NeuronCores (Bass/Tile, SPMD data-parallel).

Strategy:
  - Data-parallel over batch: each of 8 cores handles 128 of 1024 rows.
  - Embedding tables replicated to every core; device-side indirect-DMA gather.
  - Horizontal convs computed as shifted PSUM-accumulated matmuls:
      out[(i,f)-tile, (b,p)] += w[i,f,dh,:]^T @ E^T[:, b*50+p+dh]
    with (i,f) packed 8 filter-heights x 16 filters = 128 stationary columns,
    position-range restricted per tile, garbage positions masked before the
    free-dim max-reduce.
  - Vertical conv folded into the FC: G[(l,d),d'] = sum_v vfilter[l,v]*fc_w[v*D+d,d']
    is precomputed on device, then z = relu(E_flat @ G + o_h @ fc_w_h + fc_b)
    accumulated as 58 K-tile matmuls into one PSUM bank.
  - fp32 data, fp32r matmul dtype (full-rate streaming at N>=256).
"""

import os
import sys

import numpy as np

for _p in ("/opt/trn_rl_repo",):
    if os.path.isdir(_p) and _p not in sys.path:
        sys.path.append(_p)

import concourse.bass as bass
import concourse.tile as tile
import concourse.mybir as mybir
from concourse import bacc
from concourse.bass_utils import run_bass_kernel_spmd
from concourse.masks import make_identity

B, L, D = 1024, 50, 128
NV, NH = 8, 16
NU, NI = 100000, 100000
NCORES = 8
BL = B // NCORES  # 128 batch rows per core

F32 = mybir.dt.float32
F32R = mybir.dt.float32r
BF16 = mybir.dt.bfloat16
I32 = mybir.dt.int32
AF = mybir.ActivationFunctionType
ALU = mybir.AluOpType

# (i,f)-row tiles: tile t covers filter heights i in [8t, 8t+ni), 16 filters each.
TILES = []
_slot = 0
for _t in range(7):
    _i0 = 8 * _t
    _ni = min(8, L - _i0)
    _H = min(_i0 + 8, L)        # dh range needed by the tile (max h in tile)
    _P = L - _i0                # union of valid output positions in the tile
    _nb = min(512 // _P, BL)    # batch rows per PSUM chunk (<=512 fp32/bank)
    TILES.append(dict(t=_t, i0=_i0, ni=_ni, H=_H, P=_P, nb=_nb, slot0=_slot))
    _slot += _H
NSLOT = _slot  # 218


def _build():
    nc = bacc.Bacc("TRN2", target_bir_lowering=False, debug=False, num_devices=NCORES)

    item_seq = nc.dram_tensor("item_seq", [BL, L], I32, kind="ExternalInput").ap()
    user_ids = nc.dram_tensor("user_ids", [BL, 1], I32, kind="ExternalInput").ap()
    item_emb = nc.dram_tensor("item_emb", [NI, D], F32, kind="ExternalInput").ap()
    user_emb = nc.dram_tensor("user_emb", [NU, D], F32, kind="ExternalInput").ap()
    w_conv = nc.dram_tensor("w_conv", [NSLOT, D, 128], BF16, kind="ExternalInput").ap()
    maskt = nc.dram_tensor("maskt", [7, 128, L], F32, kind="ExternalInput").ap()
    hb_r = nc.dram_tensor("hb_r", [7, 128, 1], F32, kind="ExternalInput").ap()
    vfilt = nc.dram_tensor("vfilt", [L, NV], F32, kind="ExternalInput").ap()
    fc_wv = nc.dram_tensor("fc_wv", [NV * D, D], F32R, kind="ExternalInput").ap()
    fc_wh = nc.dram_tensor("fc_wh", [NH * L, D], BF16, kind="ExternalInput").ap()
    fc_b = nc.dram_tensor("fc_b", [1, D], BF16, kind="ExternalInput").ap()
    out = nc.dram_tensor("out", [BL, 2 * D], F32, kind="ExternalOutput").ap()

    with tile.TileContext(nc) as tc:
        with (
            tc.tile_pool(name="pers", bufs=1) as pers,
            tc.tile_pool(name="stage", bufs=1) as stage,
            tc.tile_pool(name="wpool", bufs=2) as wpool,
            tc.tile_pool(name="small", bufs=2) as small,
            tc.tile_pool(name="pmm", bufs=4, space="PSUM") as pmm,
            tc.tile_pool(name="pmisc", bufs=2, space="PSUM") as pmisc,
            tc.tile_pool(name="pz", bufs=1, space="PSUM") as pz,
            tc.tile_pool(name="pwarm", bufs=1, space="PSUM") as pwarm,
            tc.tile_pool(name="dpool", bufs=1, space="DRAM") as dpool,
        ):
            idn = pers.tile([128, 128], F32)
            make_identity(nc, idn[:])

            # ---- index loads + embedding gathers ----------------------------
            seq_sb = pers.tile([BL, L], I32)
            nc.sync.dma_start(out=seq_sb[:], in_=item_seq)
            uid_sb = pers.tile([BL, 1], I32)
            nc.sync.dma_start(out=uid_sb[:], in_=user_ids)

            pu_sb = pers.tile([BL, D], F32)
            nc.gpsimd.indirect_dma_start(
                out=pu_sb[:],
                out_offset=None,
                in_=user_emb,
                in_offset=bass.IndirectOffsetOnAxis(ap=uid_sb[:, 0:1], axis=0),
            )

            ebl = stage.tile([BL, L * D], F32)  # [b, (l,d)] gathered rows
            for l in range(L):
                nc.gpsimd.indirect_dma_start(
                    out=ebl[:, l * D:(l + 1) * D],
                    out_offset=None,
                    in_=item_emb,
                    in_offset=bass.IndirectOffsetOnAxis(ap=seq_sb[:, l:l + 1], axis=0),
                )

            # ---- E^T build: [d, b*50+l], 129 b-blocks (last = zero pad) -----
            et = pers.tile([128, 129 * L], BF16)
            et_ap = et[:]
            et3 = et_ap.rearrange("p (b l) -> p b l", l=L)
            zpad = stage.tile([128, L], F32, tag="zpad")
            nc.gpsimd.memset(zpad[:], 0.0)
            nc.vector.tensor_copy(out=et3[:, BL, :], in_=zpad[:])  # zero pad block
            for l in range(L):
                tp = pmisc.tile([128, 128], F32, tag="mps")
                nc.tensor.transpose(out=tp[:], in_=ebl[:, l * D:(l + 1) * D], identity=idn[:])
                nc.vector.tensor_copy(out=et3[:, 0:BL, l], in_=tp[:])

            def et_cols(col0, step, cnt, inner=None):
                """E^T column AP: partitions d, free [[step, cnt], (1, inner)]."""
                ap = [et_ap.ap[0], [step, cnt]]
                if inner is not None:
                    ap.append([1, inner])
                return bass.AP(tensor=et_ap.tensor, offset=et_ap.offset + col0, ap=ap)

            # ---- G precompute: G[(l,d),d'] = sum_v vf[l,v] fc_w[(v,d),d'] ---
            vf_sb = stage.tile([L, NV], F32)
            nc.sync.dma_start(out=vf_sb[:], in_=vfilt)
            vfT_ps = pmisc.tile([NV, L], F32, tag="mps")
            nc.tensor.transpose(out=vfT_ps[:], in_=vf_sb[:], identity=idn[0:L, 0:L])
            vfT = stage.tile([NV, L], F32R)
            nc.vector.tensor_copy(out=vfT[:], in_=vfT_ps[:])

            gdram = dpool.tile([L, D * D], BF16)
            fcwv_view = fc_wv.rearrange("(v d) e -> v (d e)", v=NV)
            for half in range(2):
                fcwv = stage.tile([NV, D * D // 2], F32R, tag="fcwv")
                nc.sync.dma_start(
                    out=fcwv[:], in_=fcwv_view[:, half * 8192:(half + 1) * 8192]
                )
                for j in range(16):
                    gps = pmisc.tile([L, 512], F32, tag="mps")
                    nc.tensor.matmul(
                        out=gps[:],
                        lhsT=vfT[:],
                        rhs=fcwv[:, j * 512:(j + 1) * 512],
                        start=True,
                        stop=True,
                    )
                    gsb = small.tile([L, 512], BF16, tag="gsb")
                    nc.vector.tensor_copy(out=gsb[:], in_=gps[:])
                    nc.sync.dma_start(
                        out=gdram[:, half * 8192 + j * 512:half * 8192 + (j + 1) * 512],
                        in_=gsb[:],
                    )
            # reshape to [d, (l, d')] for FC rhs tiles
            g_sb = pers.tile([128, L * D], BF16)
            nc.sync.dma_start(
                out=g_sb[:].rearrange("p (l e) -> p l e", l=L),
                in_=gdram[:].rearrange("l (d e) -> d l e", d=D),
            )

            # ---- FC part 1: z += E_flat @ G (independent of the convs; runs
            # in the gather/startup window and shortens the kernel tail) -----
            zps = pz.tile([BL, D], F32, tag="zps")
            for l in range(L):
                nc.tensor.matmul(
                    out=zps[:],
                    lhsT=et_cols(l, L, BL),
                    rhs=g_sb[:, l * D:(l + 1) * D],
                    start=(l == 0),
                    stop=False,
                )

            # ---- horizontal convs ------------------------------------------
            ohT = []
            for ti in TILES:
                t, H, P, nb, slot0 = ti["t"], ti["H"], ti["P"], ti["nb"], ti["slot0"]
                oh_t = pers.tile([128, BL], F32, tag=f"ohT{t}")
                ohT.append(oh_t)

                wt = wpool.tile([128, H * 128], BF16, tag="wconv")
                nc.sync.dma_start(
                    out=wt[:].rearrange("d (s m) -> d s m", s=H),
                    in_=w_conv[slot0:slot0 + H, :, :].rearrange("s d m -> d s m"),
                )
                mk = small.tile([128, P], F32, tag="mask")
                nc.sync.dma_start(out=mk[:], in_=maskt[t, :, 0:P])
                hb = small.tile([128, 1], F32, tag="hb")
                nc.sync.dma_start(out=hb[:], in_=hb_r[t])

                b0 = 0
                while b0 < BL:
                    nbc = min(nb, BL - b0)
                    ps = pmm.tile([128, nbc, P], F32, tag="cps")
                    for dh in range(H):
                        nc.tensor.matmul(
                            out=ps[:],
                            lhsT=wt[:, dh * 128:(dh + 1) * 128],
                            rhs=et_cols(b0 * L + dh, L, nbc, P),
                            start=(dh == 0),
                            stop=(dh == H - 1),
                        )
                    # mask invalid positions, then max over p (innermost axis)
                    nc.vector.tensor_tensor(
                        out=ps[:],
                        in0=ps[:],
                        in1=mk[:].unsqueeze(1).to_broadcast((128, nbc, P)),
                        op=ALU.add,
                    )
                    nc.vector.reduce_max(
                        out=oh_t[:, b0:b0 + nbc], in_=ps[:], axis=mybir.AxisListType.X
                    )
                    b0 += nbc
                # o_h = relu(max + bias), bias per (i,f) partition row
                nc.scalar.activation(out=oh_t[:], in_=oh_t[:], func=AF.Relu, bias=hb[:])

            # ---- FC: z = relu(E_flat @ G + o_h @ fc_w_h + fc_b) ------------
            for ti in TILES:
                t, rows = ti["t"], ti["ni"] * NH
                fcwh = pers.tile([128, D], BF16, tag=f"fcwh{t}")
                nc.sync.dma_start(
                    out=fcwh[0:rows, :],
                    in_=fc_wh[t * 128:t * 128 + rows, :],
                )
                ohr = pers.tile([128, BL], BF16, tag=f"ohTr{t}")
                nc.vector.tensor_copy(out=ohr[:], in_=ohT[t][:])
                nc.tensor.matmul(
                    out=zps[:],
                    lhsT=ohr[0:rows, :],
                    rhs=fcwh[0:rows, :],
                    start=False,
                    stop=False,
                )
            ones_f = pers.tile([1, BL], F32)
            nc.gpsimd.memset(ones_f[:], 1.0)
            ones = pers.tile([1, BL], BF16)
            nc.vector.tensor_copy(out=ones[:], in_=ones_f[:])
            fcb_sb = pers.tile([1, D], BF16)
            nc.sync.dma_start(out=fcb_sb[:], in_=fc_b)
            nc.tensor.matmul(
                out=zps[:],
                lhsT=ones[:],
                rhs=fcb_sb[:],
                start=False,
                stop=True,
            )
            z_sb = pers.tile([BL, D], F32)
            nc.scalar.activation(out=z_sb[:], in_=zps[:], func=AF.Relu)

            nc.sync.dma_start(out=out[:, 0:D], in_=z_sb[:])
            nc.sync.dma_start(out=out[:, D:2 * D], in_=pu_sb[:])


    nc.compile()
    return nc


_CACHE = None


def _get_compiled():
    global _CACHE
    if _CACHE is None:
        _CACHE = _build()
    return _CACHE


def _prep_weights(hconv_w, hconv_b):
    """Masked, transposed conv weights + position masks + per-row bias."""
    w_all = np.zeros((NSLOT, D, 128), np.float32)
    mask = np.full((7, 128, L), -1e30, np.float32)
    hb = np.zeros((7, 128, 1), np.float32)
    for ti in TILES:
        t, i0, ni, H, slot0 = ti["t"], ti["i0"], ti["ni"], ti["H"], ti["slot0"]
        for di in range(ni):
            i = i0 + di
            m0 = di * NH
            # filter i has height h=i+1: rows dh<=i, valid positions p<50-i
            w_all[slot0:slot0 + i + 1, :, m0:m0 + NH] = (
                hconv_w[i, :, 0:i + 1, :].transpose(1, 2, 0)  # [h, d, f]
            )
            mask[t, m0:m0 + NH, 0:L - i] = 0.0
            hb[t, m0:m0 + NH, 0] = hconv_b[i]
    return w_all, mask, hb


def _make_in_maps(user_ids, item_seq, user_emb, item_emb, vfilter, hconv_w,
                  hconv_b, fc_w, fc_b):
    uid = np.ascontiguousarray(np.asarray(user_ids).astype(np.int32).reshape(B, 1))
    iseq = np.ascontiguousarray(np.asarray(item_seq).astype(np.int32))
    ue = np.ascontiguousarray(np.asarray(user_emb, dtype=np.float32))
    ie = np.ascontiguousarray(np.asarray(item_emb, dtype=np.float32))
    vf = np.ascontiguousarray(np.asarray(vfilter, dtype=np.float32))
    hw = np.ascontiguousarray(np.asarray(hconv_w, dtype=np.float32))
    hbias = np.asarray(hconv_b, dtype=np.float32)
    import ml_dtypes
    fw = np.asarray(fc_w, dtype=np.float32)
    fwv = np.ascontiguousarray(fw[0:NV * D])
    fwh = np.ascontiguousarray(fw[NV * D:].astype(ml_dtypes.bfloat16))
    fb = np.ascontiguousarray(
        np.asarray(fc_b, dtype=np.float32).reshape(1, D).astype(ml_dtypes.bfloat16))

    w_all, mask, hb = _prep_weights(hw, hbias)
    w_all = w_all.astype(ml_dtypes.bfloat16)

    in_maps = []
    for c in range(NCORES):
        sl = slice(c * BL, (c + 1) * BL)
        in_maps.append({
            "item_seq": iseq[sl],
            "user_ids": uid[sl],
            "item_emb": ie,
            "user_emb": ue,
            "w_conv": w_all,
            "maskt": mask,
            "hb_r": hb,
            "vfilt": vf,
            "fc_wv": fwv,
            "fc_wh": fwh,
            "fc_b": fb,
        })

    return in_maps


def kernel(user_ids, item_seq, user_emb, item_emb, vfilter, hconv_w, hconv_b,
           fc_w, fc_b):
    nc = _get_compiled()
    in_maps = _make_in_maps(user_ids, item_seq, user_emb, item_emb, vfilter,
                            hconv_w, hconv_b, fc_w, fc_b)
    res = run_bass_kernel_spmd(nc, in_maps, core_ids=list(range(NCORES)))
    return np.concatenate([res.results[c]["out"] for c in range(NCORES)], axis=0)

